# revision 83
# baseline (speedup 1.0000x reference)
"""Multi-head causal self-attention (B=2, T=4096, C=512, H=8) on 8 trn2 cores.

Sharding: 16 (batch, head) pairs -> 2 heads per core. Core c handles batch
c//4, heads {2*(c%4), 2*(c%4)+1}. All matmuls run in bf16 (1 cycle/row at
any free size), so causal trimming is 128-block granular and the PV matmul
is flipped: the exp'd scores [k, q] are the stationary operand while V
streams 65 columns per k-block, a single matmul per (block, q-sub-block)
whose 65th column is an all-ones vector accumulating the softmax row-sums
in the same instruction (V is stored per-block/per-head with the ones
column embedded). exp is split between ACT (true exp) and DVE (Schraudolph
bit-trick: bf16 bits = int16(s*128*log2e + 16253.5), ~1% error), balanced
by an emission-time cost ledger; exps are emitted before drip-fed copies
so copies queue behind them, and PSUM-staging copies ride on the engine
that just ran an exp (its next exp is furthest away). Triangular causal
masks run on Pool (SBUF-only), keeping them off ACT/DVE, and diagonal
k-blocks go first so masks stay off the PE critical path. The three
engines are co-critical (~117us each of 140): PE work is floored by the
causal score stream (128 values/cycle), and all exp+staging traffic must
flow through ACT/DVE since only they can read PSUM. Segments are
software-pipelined ACROSS boundaries: the tail PV matmuls of a segment
stay queued while the next segment's diagonal scores issue, and the
normalization (DVE reciprocal of row sums, ACT/DVE staging copy, Pool
per-row scale, double-buffered PSUM accumulators) is emitted one segment
late, which keeps the PE from draining against the final exp's latency.
V-projection for a whole 512-chunk lands in one PSUM tile and stages with
a single copy; o [q,d] -> [d,q] via PE transposes (two per staging copy)
for the row-sliced output projection; per-chunk partials stream out in
bf16 on rotating DMA queues in the last chunk, and the host sums the 4
partials per batch, folding the V-bias and output bias in once per batch.
fp8 DoubleRow matmuls (0.5 cyc/row) were evaluated and rejected: e4m3
quantization of V alone costs ~2.7e-2 max-rel-err (sharp softmax rows
pass V's quantization error straight through), over the 2e-2 gate.
"""

import numpy as np
import ml_dtypes

import concourse.bass as bass
import concourse.mybir as mybir
import concourse.tile as tile
from concourse import bacc
from concourse.bass_utils import run_bass_kernel_spmd

B, T, C, H, D = 2, 4096, 512, 8, 64
NCORES = 8
SCALE = 1.0 / np.sqrt(D)

F32 = mybir.dt.float32
BF16 = mybir.dt.bfloat16
I16 = mybir.dt.int16
BF = ml_dtypes.bfloat16

# Schraudolph exp in bf16 bits: i16 = round(s*A + Bc); bf16(i16) ~ exp(s)
SCH_A = 128.0 / np.log(2.0)
SCH_B = 127.0 * 128.0 - 2.5

TRACE = False
LAST_RESULT = None
_NC = None

_ACT, _DVE, _POOL = "act", "dve", "pool"


def _build(qk_bias=True):
    nc = bacc.Bacc()

    xt = nc.declare_dram_parameter("xt", [4, 128, T], BF16, isOutput=False)
    wq = nc.declare_dram_parameter("wq", [4, 128, 128], BF16, isOutput=False)
    wk = nc.declare_dram_parameter("wk", [4, 128, 128], BF16, isOutput=False)
    wv = nc.declare_dram_parameter("wv", [4, 128, 128], BF16, isOutput=False)
    wout = nc.declare_dram_parameter("wout", [128, 4, 128], BF16,
                                     isOutput=False)
    sblob = nc.declare_dram_parameter("sblob", [128, 384], BF16,
                                      isOutput=False)
    fblob = nc.declare_dram_parameter("fblob", [128, 2], F32, isOutput=False)
    out_t = nc.declare_dram_parameter("out_t", [C, T], BF16, isOutput=True)

    busy = {_ACT: 0.0, _DVE: 0.0, _POOL: 0.0}
    RATE = {_ACT: 0.8333, _DVE: 1.0417, _POOL: 1.39}
    FIX = {_ACT: 185.0, _DVE: 125.0, _POOL: 131.0}

    def ledger(eng, cols):
        busy[eng] += cols * RATE[eng] + FIX[eng]

    def pick(cols, engines):
        return min(engines, key=lambda e: busy[e] + cols * RATE[e] + FIX[e])

    with tile.TileContext(nc) as tc:
        with (
            tc.tile_pool(name="w", bufs=1) as w,
            tc.tile_pool(name="sb", bufs=4) as sb,
            tc.tile_pool(name="sbA", bufs=8) as sbA,
            tc.tile_pool(name="psS", bufs=4, space="PSUM") as psS,
            tc.tile_pool(name="psO", bufs=2, space="PSUM") as psO,
            tc.tile_pool(name="psX", bufs=2, space="PSUM") as psX,
        ):
            # ---- persistent tiles ----
            wq_s = w.tile([128, 4, 128], BF16)
            wk_s = w.tile([128, 4, 128], BF16)
            wv_s = w.tile([128, 4, 128], BF16)
            wout_s = w.tile([128, 4, 128], BF16)
            sblob_s = w.tile([128, 384], BF16)
            mask2_s = sblob_s[:, 0:256]    # two triu copies side by side
            ident_s = sblob_s[:, 256:384]
            fblob_s = w.tile([128, 2], F32)
            qb_s = fblob_s[:, 0:1]
            kb_s = fblob_s[:, 1:2]

            xt_s = w.tile([128, 4, T], BF16)
            qt_s = w.tile([128, T], BF16)  # partitions: [h0 dims | h1 dims]
            kt_s = w.tile([128, T], BF16)
            # V per k-block: [block, head, 64 dims + ones col]
            v8r = w.tile([128, 32, 2, 65], BF16)

            # ---- engine dispatch helpers ----
            last_exp_eng = [_DVE]

            def emit_exp(eng, at2, sc2, c0, c1):
                last_exp_eng[0] = eng
                cols = c1 - c0
                if eng == _ACT:
                    nc.scalar.activation(
                        at2[:, c0:c1], sc2[:, c0:c1],
                        mybir.ActivationFunctionType.Exp,
                    )
                else:
                    nc.vector.tensor_scalar(
                        at2.bitcast(I16)[:, c0:c1], sc2[:, c0:c1],
                        SCH_A, SCH_B,
                        mybir.AluOpType.mult, mybir.AluOpType.add,
                    )
                ledger(eng, cols)

            def emit_mask1(at2, c0):
                nc.gpsimd.tensor_tensor(
                    at2[:, c0:c0 + 128], at2[:, c0:c0 + 128],
                    mask2_s[:, 0:128], mybir.AluOpType.mult,
                )
                ledger(_POOL, 128)

            def copy_op(dst, src, cols, scalar=None, engines=None):
                # NOTE: Pool/GPSIMD cannot access PSUM; psum-reading copies
                # may only run on ACT (plain Copy) or DVE (tensor_scalar).
                # Prefer the engine that just ran an exp: its next exp is two
                # pair-periods away, so a copy there never delays the exp
                # chain that the PE's score pipeline is gated on.
                if engines is None:
                    pref = last_exp_eng[0]
                    other = _ACT if pref == _DVE else _DVE
                    if busy[pref] - busy[other] < 3000.0:
                        eng = pref
                    else:
                        eng = other
                else:
                    eng = pick(cols, engines)
                e = {_DVE: nc.vector, _POOL: nc.gpsimd,
                     _ACT: nc.scalar}[eng]
                if eng == _ACT:
                    assert scalar is None
                    nc.scalar.activation(dst, src,
                                         mybir.ActivationFunctionType.Copy)
                elif scalar is None:
                    e.tensor_copy(dst, src)
                else:
                    e.tensor_scalar(dst, src, scalar, None,
                                    mybir.AluOpType.mult)
                ledger(eng, cols)

            # ---- projection pieces (drip-fed under attention) ----
            def proj_qk(g, which):
                ws, dst, bias = ((wq_s, qt_s, qb_s) if which == "q"
                                 else (wk_s, kt_s, kb_s))
                sl = bass.ts(g, 512)
                pp = psX.tile([128, 512], F32, tag="x")
                for ch in range(4):
                    nc.tensor.matmul(
                        pp, ws[:, ch, :], xt_s[:, ch, sl],
                        start=(ch == 0), stop=(ch == 3),
                    )
                if qk_bias:
                    nc.vector.tensor_scalar(
                        dst[:, sl], pp, 1.0, bias,
                        mybir.AluOpType.mult, mybir.AluOpType.add,
                    )
                    ledger(_DVE, 512)
                else:
                    # zero biases (always true for this problem's inputs):
                    # a plain copy, eligible for either engine
                    copy_op(dst[:, sl], pp, 512)

            def proj_v4(g):
                # all 4 k-blocks of chunk g -> one PSUM tile, ONE copy
                pv4 = psX.tile([128, 4, 2, 64], F32, tag="x")
                for t4 in range(4):
                    tt = g * 4 + t4
                    for ch in range(4):
                        nc.tensor.matmul(
                            pv4[:, t4, :, :],
                            xt_s[:, ch, bass.ts(tt, 128)], wv_s[:, ch, :],
                            start=(ch == 0), stop=(ch == 3),
                        )
                copy_op(v8r[:, 4 * g:4 * g + 4, :, 0:64], pv4, 512)

            proj_pending = []  # (chunk, kind, fn)
            deferred = []

            def queue_proj(g):
                sl = bass.ts(g, 512)
                nc.sync.dma_start(out=xt_s[:, :, sl],
                                  in_=xt.rearrange("c p t -> p c t")[:, :, sl])
                for which in ("q", "k"):
                    proj_pending.append(
                        (g, "qk", lambda g=g, w_=which: proj_qk(g, w_)))
                proj_pending.append((g, "v4", lambda g=g: proj_v4(g)))

            def force_proj(g, kinds):
                # pop pending pieces (in order) whose chunk <= g and whose
                # kind is in kinds; stop at the first non-matching entry to
                # preserve emission order within a chunk
                while proj_pending and proj_pending[0][0] <= g and \
                        proj_pending[0][1] in kinds:
                    proj_pending.pop(0)[2]()

            drip_tick = [0]

            dma_pending = []
            last_chunk = [False]

            def drip():
                drip_tick[0] += 1
                if dma_pending and drip_tick[0] % 2 == 1:
                    dma_pending.pop(0)()
                if proj_pending:
                    proj_pending.pop(0)[2]()
                elif deferred:
                    # drain deferred at half rate to keep outproj matmuls
                    # well behind their transposed inputs (full rate in the
                    # last chunk so the tail doesn't serialize)
                    if last_chunk[0] or drip_tick[0] % 2 == 0:
                        deferred.pop(0)()

            # ---- attention ----
            pv_queue = []          # entries: (seg_id, closure)
            pending_norm = [None]  # previous segment's normalization emitter
            seg_counter = [0]

            def attn_segment(g, h, onorm_s, after_norm=None):
                seg = seg_counter[0]
                seg_counter[0] += 1
                if h == 0:
                    # q/k of this chunk gate the scores; v4 only gates the
                    # PV pops, so it can ride behind the diag pairs' scores
                    force_proj(g, ("qk",))
                hb = h * 64
                o_ps = psO.tile([128, 4, 65], F32, tag="o")
                # one PSUM bank cannot host 4 concurrent accumulation groups;
                # zero it once and accumulate group-free instead
                nc.vector.memset(o_ps, 0.0)
                ledger(_DVE, 260)
                # diag blocks first (their masks stay off the PE critical
                # path)
                js = list(range(4 * g, 4 * g + 4)) + list(range(4 * g))
                # per-qb bookkeeping: block j contributes to qb iff
                # j <= 4g + qb; find the last emitted j per qb for stop flags
                lastj = {}
                for idx, j in enumerate(js):
                    for qb in range(4):
                        if j <= 4 * g + qb:
                            lastj[qb] = idx

                for idx, j in enumerate(js):
                    at_s = sbA.tile([128, 512], BF16, tag="attn")
                    sc_ps = psS.tile([128, 512], F32, tag="sc")
                    d = j - 4 * g
                    q0 = max(d, 0) * 128
                    cols = 512 - q0
                    nc.tensor.matmul(
                        sc_ps[:, q0:512],
                        kt_s[hb:hb + 64, bass.ts(j, 128)],
                        qt_s[hb:hb + 64, g * 512 + q0:(g + 1) * 512],
                        start=True, stop=True,
                    )
                    # emit exp before drip() so drip-fed copies queue
                    # BEHIND it on its engine
                    eng = pick(cols, (_ACT, _DVE))
                    emit_exp(eng, at_s, sc_ps, q0, 512)
                    if d >= 0:
                        # ride the mask in-order behind a DVE exp (no
                        # cross-engine hop); otherwise Pool applies it
                        if eng == _DVE:
                            nc.vector.tensor_tensor(
                                at_s[:, q0:q0 + 128], at_s[:, q0:q0 + 128],
                                mask2_s[:, 0:128], mybir.AluOpType.mult,
                            )
                            ledger(_DVE, 128)
                        else:
                            emit_mask1(at_s, q0)
                    drip()

                    def pv(j=j, d=d, at_s=at_s, idx=idx, h=h, g=g):
                        for qb in range(max(d, 0), 4):
                            nc.tensor.matmul(
                                o_ps[:, qb, :],
                                at_s[:, 128 * qb:128 * qb + 128],
                                v8r[:, j, h, :],
                                start=False,
                                stop=(lastj[qb] == idx),
                                skip_group_check=True,
                            )
                    if idx == 3:
                        # segment boundary: the new diag blocks' scores have
                        # given the previous segment's final exp time to
                        # land; now drain its leftover PVs and emit its
                        # deferred normalization (must precede any deferred
                        # transpose pops that read onorm)
                        while pv_queue and pv_queue[0][0] < seg:
                            pv_queue.pop(0)[1]()
                        if pending_norm[0] is not None:
                            pending_norm[0]()
                            pending_norm[0] = None
                        force_proj(g, ("qk", "v4"))
                    pv_queue.append((seg, pv))
                    if len(pv_queue) > 6:
                        pv_queue.pop(0)[1]()

                fast = (g == 7 and h == 1)

                def norm(o_ps=o_ps, onorm_s=onorm_s, hb=hb,
                         after_norm=after_norm, fast=fast):
                    # DVE recip (psum), ACT/DVE stage o to SBUF, Pool
                    # (SBUF-only) applies the per-row reciprocal. The final
                    # segment's applies are latency-critical (nothing left
                    # to overlap), so they run on ACT/DVE instead.
                    rec_s = sb.tile([128, 4], F32, tag="rec")
                    with nc.allow_low_precision(reason="softmax denom"):
                        nc.vector.reciprocal(rec_s, o_ps[:, :, 64])
                    ledger(_DVE, 4)
                    osb_s = sb.tile([128, 4, 64], BF16, tag="osb")
                    copy_op(osb_s, o_ps[:, :, 0:64], 256)
                    for qb in range(4):
                        if fast and qb % 2 == 0:
                            nc.vector.tensor_scalar(
                                onorm_s[:, qb, hb:hb + 64], osb_s[:, qb, :],
                                rec_s[:, qb:qb + 1], None,
                                mybir.AluOpType.mult,
                            )
                            ledger(_DVE, 64)
                        elif fast:
                            nc.scalar.activation(
                                onorm_s[:, qb, hb:hb + 64], osb_s[:, qb, :],
                                mybir.ActivationFunctionType.Copy,
                                0.0, rec_s[:, qb:qb + 1],
                            )
                            ledger(_ACT, 64)
                        else:
                            nc.gpsimd.tensor_scalar(
                                onorm_s[:, qb, hb:hb + 64], osb_s[:, qb, :],
                                rec_s[:, qb:qb + 1], None,
                                mybir.AluOpType.mult,
                            )
                            ledger(_POOL, 64)
                    if after_norm is not None:
                        # deferred transposes/outproj for this chunk may only
                        # be queued once both heads' onorm writes are emitted
                        after_norm()
                pending_norm[0] = norm

            def outproj_m(g, onormT_s, oc_s, m):
                op_ps = psX.tile([128, 512], F32, tag="x")
                nc.tensor.matmul(
                    op_ps, wout_s[:, m, :], onormT_s,
                    start=True, stop=True,
                )
                copy_op(oc_s[:, m, :], op_ps, 512)
                if last_chunk[0] or m == 3:
                    # one store per chunk normally; per-m on rotating DMA
                    # queues in the last chunk so the final stores overlap
                    # the remaining copies and each other
                    m0 = m if last_chunk[0] else 0
                    q_ = [nc.sync, nc.scalar, nc.gpsimd, nc.scalar][m] \
                        if last_chunk[0] else nc.sync
                    dma_pending.append(
                        lambda g=g, oc_s=oc_s, m0=m0, m=m, q_=q_:
                        q_.dma_start(
                            out=out_t.rearrange(
                                "(m p) t -> p m t",
                                m=4)[:, m0:m + 1, bass.ts(g, 512)],
                            in_=oc_s[:, m0:m + 1, :]))

            # ---- startup: interleave DMAs across queues, start projecting
            # as soon as the needed chunks land ----
            # HWDGE descriptor generation is a single shared serial
            # resource: issue the latency-critical transfers first (xt
            # chunk 0, then wq/wk), and push the non-gating blobs through
            # the separate SWDGE path on the Pool queue
            sl0 = bass.ts(0, 512)
            nc.sync.dma_start(out=xt_s[:, :, sl0],
                              in_=xt.rearrange("c p t -> p c t")[:, :, sl0])
            nc.scalar.dma_start(out=wq_s, in_=wq.rearrange("c p m -> p c m"))
            nc.scalar.dma_start(out=wk_s, in_=wk.rearrange("c p m -> p c m"))
            nc.scalar.dma_start(out=wv_s, in_=wv.rearrange("c p m -> p c m"))
            # memsets go FIRST on the Pool queue: each SWDGE dma_start
            # costs ~1us of descriptor generation, and the wu memset gates
            # the PE p-state warmup
            wu_s = sb.tile([128, 64], BF16, tag="wu", bufs=1)
            nc.gpsimd.memset(wu_s, 0.0)
            # ones columns for the softmax row-sums (col 64 of each v8r
            # slot) - on DVE, which is idle at t=0
            nc.vector.memset(v8r[:, :, :, 64:65], 1.0)
            nc.gpsimd.dma_start(out=fblob_s, in_=fblob[:])
            nc.gpsimd.dma_start(out=sblob_s, in_=sblob[:])
            nc.gpsimd.dma_start(out=wout_s, in_=wout[:])
            warm_s = sb.tile([1, 1], F32, tag="warm")
            nc.scalar.activation(warm_s, fblob_s[0:1, 0:1],
                                 mybir.ActivationFunctionType.Exp)
            # PE p-state warmup during the startup DMAs (cheap ap-64 matmuls)
            wu_ps = psX.tile([128, 64], F32, tag="x")
            for _ in range(64):
                nc.tensor.matmul(wu_ps[0:64, :], wu_s[:, 0:64], wu_s,
                                 start=True, stop=True)
            for which in ("q", "k"):
                proj_qk(0, which)
            proj_pending.append((0, "v4", lambda: proj_v4(0)))

            for g in range(8):
                if g < 7:
                    queue_proj(g + 1)
                else:
                    last_chunk[0] = True
                onorm_s = sb.tile([128, 4, 128], BF16, tag="onorm")
                onormT_s = sb.tile([128, 512], BF16, tag="onormT")
                oc_s = sb.tile([128, 4, 512], BF16, tag="outc")

                # transpose [q,d]->[d,q] on the PE (bf16, 128 cyc each);
                # all 4 land in one PSUM tile so a single copy stages them
                def trans2(onorm_s, onormT_s, half):
                    tr_ps = psX.tile([128, 2, 128], BF16, tag="x")
                    for i in range(2):
                        qb = 2 * half + i
                        nc.tensor.transpose(tr_ps[:, i, :],
                                            onorm_s[:, qb, :], ident_s)
                    copy_op(onormT_s[:, 256 * half:256 * half + 256],
                            tr_ps, 256)

                def queue_chunk_tail(g=g, onorm_s=onorm_s,
                                     onormT_s=onormT_s, oc_s=oc_s):
                    for half in range(2):
                        deferred.append(
                            lambda onorm_s=onorm_s, onormT_s=onormT_s,
                            half=half: trans2(onorm_s, onormT_s, half))
                    for m in range(4):
                        deferred.append(
                            lambda g=g, onormT_s=onormT_s, oc_s=oc_s, m=m:
                            outproj_m(g, onormT_s, oc_s, m))

                attn_segment(g, 0, onorm_s)
                attn_segment(g, 1, onorm_s, after_norm=queue_chunk_tail)
            while pv_queue:
                pv_queue.pop(0)[1]()
            if pending_norm[0] is not None:
                pending_norm[0]()
                pending_norm[0] = None
            while proj_pending or deferred or dma_pending:
                if proj_pending:
                    proj_pending.pop(0)[2]()
                elif deferred:
                    deferred.pop(0)()
                else:
                    dma_pending.pop(0)()
    nc.compile()
    return nc


def _pack_inputs(x, Wqkv, bqkv, Wout, bout):
    sb_host = np.zeros((128, 384), dtype=np.float32)
    triu = np.triu(np.ones((128, 128), dtype=np.float32))
    sb_host[:, 0:128] = triu
    sb_host[:, 128:256] = triu
    sb_host[:, 256:384] = np.eye(128, dtype=np.float32)
    in_maps = []
    for c in range(NCORES):
        b = c // 4
        h0 = 2 * (c % 4)
        cq = h0 * 64
        xt = np.ascontiguousarray(x[b].T.reshape(4, 128, T))
        wq_c = np.ascontiguousarray(
            (Wqkv[:, cq:cq + 128] * SCALE).reshape(4, 128, 128))
        wk_c = np.ascontiguousarray(
            Wqkv[:, 512 + cq:512 + cq + 128].reshape(4, 128, 128))
        wv_c = np.ascontiguousarray(
            Wqkv[:, 1024 + cq:1024 + cq + 128].reshape(4, 128, 128))
        wout_c = np.ascontiguousarray(
            Wout[cq:cq + 128, :].reshape(128, 4, 128))
        fblob = np.zeros((128, 2), dtype=np.float32)
        fblob[:, 0] = bqkv[cq:cq + 128] * SCALE
        fblob[:, 1] = bqkv[512 + cq:512 + cq + 128]
        in_maps.append({
            "xt": xt.astype(BF),
            "wq": wq_c.astype(BF), "wk": wk_c.astype(BF),
            "wv": wv_c.astype(BF), "wout": wout_c.astype(BF),
            "sblob": sb_host.astype(BF), "fblob": fblob,
        })
    return in_maps


def kernel(x, Wqkv, bqkv, Wout, bout):
    global _NC, LAST_RESULT
    x = np.asarray(x, dtype=np.float32)
    Wqkv = np.asarray(Wqkv, dtype=np.float32)
    bqkv = np.asarray(bqkv, dtype=np.float32)
    Wout = np.asarray(Wout, dtype=np.float32)
    bout = np.asarray(bout, dtype=np.float32)

    if _NC is None:
        _NC = _build(qk_bias=bool(np.any(bqkv[:1024] != 0.0)))
    in_maps = _pack_inputs(x, Wqkv, bqkv, Wout, bout)
    res = run_bass_kernel_spmd(_NC, in_maps, list(range(NCORES)), trace=TRACE)
    LAST_RESULT = res
    # v-bias contribution (sum_k attn = 1) + output bias, applied once/batch
    base = (bqkv[1024:] @ Wout + bout).astype(np.float32)
    out = np.zeros((B, T, C), dtype=np.float32)
    out += base
    for c in range(NCORES):
        out[c // 4] += res.results[c]["out_t"].astype(np.float32).T
    return out
